# revision 11
# baseline (speedup 1.0000x reference)
"""Trainium2 Bass kernel for nn_Euler: 512-step Euler integration of a
2-layer tanh MLP, data-parallel over 8 NeuronCores (batch 1024 -> 128/core).

v2: all data movement on device + cached-jit dispatch.
  - Inputs are raw contiguous batch-slices (control_inputs (B,L,U) f32,
    initial_state (B,S) f32) -- zero host-side prep for the big tensors.
  - Device pre-pass transposes u to (L,U,BLOC) and splits bf16 hi/lo into
    internal DRAM; prologue transposes s0 via the PE array.
  - Main loop: mm1 in bf16 hi/lo 3-term, tanh, mm2 fp32, Euler update;
    each new state is PE-transposed to batch-major and written f16 to a
    (BLOC, L, S) output -- host gather is a reshape + astype only.
  - The jax.jit(shard_map(bass_exec)) callable is built ONCE and cached;
    output placeholder buffers live on device across calls.
"""

import time
import numpy as np
from contextlib import ExitStack

SCALE_PREDIV = True  # wire scales are m/127 (host multiplies directly)
B, L, S, U, H = 1024, 512, 64, 32, 512
DT = 0.1
NCORES = 8
BLOC = B // NCORES  # 128
KZ = S + U + 1      # 97 (state + control + bias row)
NCH = H // 128      # 4 H-chunks

_COMPILED = {}


def _build(nsteps):
    import concourse.bass as cbass
    import concourse.bacc as bacc
    import concourse.tile as tile
    import concourse.mybir as mybir

    F32 = mybir.dt.float32
    BF16 = mybir.dt.bfloat16  # hi/lo split dtype
    F16 = mybir.dt.float16    # output wire dtype
    TANH = mybir.ActivationFunctionType.Tanh
    COPY = mybir.ActivationFunctionType.Copy
    ADD = mybir.AluOpType.add
    SUB = mybir.AluOpType.subtract
    ds = cbass.ds

    nc = bacc.Bacc("TRN2", target_bir_lowering=False, debug=False,
                   num_devices=NCORES)

    I8 = mybir.dt.int8
    OBLK = 8                    # output steps per packed row / flush
    NBLK = nsteps // OBLK
    ROWB = OBLK * S + 2 * OBLK  # 528: 8x64 int8 q + 8 f16 scales as raw bytes

    # external inputs (per-core shapes; raw slices of the full arrays)
    s0_d = nc.dram_tensor("s0", [BLOC, S], F32, kind="ExternalInput").ap()
    # u arrives f16 on the wire (halves H2D, ~f32 precision); split bf16 hi/lo on device
    u_d = nc.dram_tensor("u", [BLOC, nsteps, U], F16, kind="ExternalInput").ap()
    eye_d = nc.dram_tensor("eye", [128, 128], F32, kind="ExternalInput").ap()
    w1hi_d = nc.dram_tensor("w1hi", [KZ, H], BF16, kind="ExternalInput").ap()
    w1lo_d = nc.dram_tensor("w1lo", [KZ, H], BF16, kind="ExternalInput").ap()
    w2_d = nc.dram_tensor("w2", [NCH, 128, S], F32, kind="ExternalInput").ap()
    b2_d = nc.dram_tensor("b2row", [1, S], F32, kind="ExternalInput").ap()
    # internal transposed control inputs, (block, sub-step) shaped so the main
    # loop can index them with a block-granular induction variable; one padding
    # block at the end (only its sub-row 0 is ever prefetched)
    uhiT4_d = nc.dram_tensor("uhiT", [NBLK + 1, OBLK, U, BLOC], BF16, kind="Internal").ap()
    uloT4_d = nc.dram_tensor("uloT", [NBLK + 1, OBLK, U, BLOC], BF16, kind="Internal").ap()
    uhiT_d = uhiT4_d.rearrange("a b u c -> (a b) u c")
    uloT_d = uloT4_d.rearrange("a b u c -> (a b) u c")
    # batch-major int8 output with the per-(batch,step) f16 scales PACKED into
    # the same tensor (one D2H fetch; a separate 1MB fetch costs ~0.1s of fixed
    # RPC overhead on the axon tunnel).  Row layout per (batch, 8-step block):
    # [8*64 int8 q values][8 f16 scales as 16 raw bytes].  States are sent as
    # q = round_hw(s * 127/m) with m = max_s |s|; host reconstructs
    # s = q * m/127.  Quantization error <= m/127 <= absmax/127 worst-case.
    out_d = nc.dram_tensor("out", [BLOC, NBLK, ROWB], I8, kind="ExternalOutput").ap()
    # exact f32 carry so the trajectory can be split into pipelined segments
    sout_d = nc.dram_tensor("sout", [BLOC, S], F32, kind="ExternalOutput").ap()

    # Two independent half-batch recurrences (64 lanes each) interleaved per
    # step: while one way walks its cross-engine latency chain (psum drain ->
    # DVE state update -> re-split), the PE runs the other way's matmuls.
    # This keeps the PE continuously busy (full p-state) and hides the
    # semaphore/engine-access latencies that dominate the single-stream step.
    NWAY = 2
    BW = BLOC // NWAY  # 64
    WSL = [slice(w * BW, (w + 1) * BW) for w in range(NWAY)]

    with tile.TileContext(nc) as tc, ExitStack() as ctx:
        cpool = ctx.enter_context(tc.tile_pool(name="const", bufs=1))
        spool = ctx.enter_context(tc.tile_pool(name="state", bufs=1))
        hpool = ctx.enter_context(tc.tile_pool(name="h", bufs=2))
        opool = ctx.enter_context(tc.tile_pool(name="outs", bufs=2))
        qpool = ctx.enter_context(tc.tile_pool(name="quant", bufs=2))

        # --- static weights/constants ---
        eye = cpool.tile([128, 128], F32)
        w1hi = cpool.tile([KZ, H], BF16)
        w1lo = cpool.tile([KZ, H], BF16)
        w2 = cpool.tile([128, NCH * S], F32)
        b2r = cpool.tile([1, S], F32)
        ones = cpool.tile([1, BLOC], F32)
        nc.sync.dma_start(eye[:, :], eye_d[:, :])
        nc.sync.dma_start(w1hi[:, :], w1hi_d[:, :])
        nc.sync.dma_start(w1lo[:, :], w1lo_d[:, :])
        for j in range(NCH):
            nc.sync.dma_start(w2[:, j * S:(j + 1) * S], w2_d[j, :, :])
        nc.sync.dma_start(b2r[:, :], b2_d[:, :])
        nc.vector.memset(ones[:, :], 1.0)

        # --- per-way double-buffered z (hi/lo) and state tiles ---
        zhi = [[spool.tile([KZ, BW], BF16, tag=f"zhi{w}{i}", name=f"zhi{w}{i}")
                for i in range(2)] for w in range(NWAY)]
        zlo = [[spool.tile([KZ, BW], BF16, tag=f"zlo{w}{i}", name=f"zlo{w}{i}")
                for i in range(2)] for w in range(NWAY)]
        sT = [[spool.tile([S, BW], F32, tag=f"sT{w}{i}", name=f"sT{w}{i}")
               for i in range(2)] for w in range(NWAY)]
        for w in range(NWAY):
            for i in range(2):
                nc.vector.memset(zhi[w][i][S + U:KZ, :], 1.0)  # bias row (hi = 1.0)
                nc.vector.memset(zlo[w][i][S + U:KZ, :], 0.0)  # bias row (lo = 0)

        eye16 = cpool.tile([128, 128], F16)
        nc.vector.tensor_copy(eye16[:, :], eye[:, :])

        # --- u pre-pass: (BLOC, L, U) f16 -> (L, U, BLOC) bf16 hi/lo ---
        with ExitStack() as pctx:
            prpool = pctx.enter_context(tc.tile_pool(name="pre", bufs=3))
            prps = pctx.enter_context(tc.tile_pool(name="preps", bufs=2, space="PSUM"))
            with tc.For_i(0, nsteps, 4) as pi:
                raw = prpool.tile([128, 128], F16, tag="praw")
                nc.sync.dma_start(raw[:, :], u_d[:, ds(pi, 4), :])
                pt = prps.tile([128, 128], F16, tag="ppt")
                nc.tensor.transpose(pt[:, :], raw[:, :], eye16[:, :])
                uhi = prpool.tile([128, 128], BF16, tag="puhi")
                ulo = prpool.tile([128, 128], BF16, tag="pulo")
                nc.vector.tensor_copy(uhi[:, :], pt[:, :])
                nc.vector.tensor_tensor(ulo[:, :], pt[:, :], uhi[:, :], SUB)
                nc.sync.dma_start(
                    uhiT_d[ds(pi, 4), :, :].rearrange("k u b -> (k u) b"), uhi[:, :])
                nc.sync.dma_start(
                    uloT_d[ds(pi, 4), :, :].rearrange("k u b -> (k u) b"), ulo[:, :])
            # padding row nsteps (prefetched by the last step, never used)
            upad = prpool.tile([U, BLOC], BF16, tag="ppad")
            nc.vector.memset(upad[:, :], 0.0)
            nc.sync.dma_start(uhiT_d[nsteps, :, :], upad[:, :])
            nc.sync.dma_start(uloT_d[nsteps, :, :], upad[:, :])

            # --- prologue: transpose s0 on device, seed state buffers ---
            s0raw = cpool.tile([BLOC, S], F32)
            nc.sync.dma_start(s0raw[:, :], s0_d[:, :])
            ps0 = prps.tile([128, 128], F32, tag="ppt", name="ps0")
            nc.tensor.transpose(ps0[:S, :], s0raw[:, :], eye[:, :])
            for w in range(NWAY):
                nc.vector.tensor_copy(sT[w][0][:, :], ps0[:S, WSL[w]])
                nc.vector.tensor_copy(zhi[w][0][:S, :], sT[w][0][:, :])
                nc.vector.tensor_tensor(zlo[w][0][:S, :], sT[w][0][:, :],
                                        zhi[w][0][:S, :], SUB)
                nc.sync.dma_start(zhi[w][0][S:S + U, :], uhiT_d[0, :, WSL[w]])
                nc.sync.dma_start(zlo[w][0][S:S + U, :], uloT_d[0, :, WSL[w]])

        # PSUM pools enter after the pre-pass pool has been released:
        # ph 2 tags x 2 bufs + pd 2 tags + pt 2 tags = 8 banks exactly.
        pp_h = ctx.enter_context(tc.tile_pool(name="ps_h", bufs=2, space="PSUM"))
        pp_d = ctx.enter_context(tc.tile_pool(name="ps_d", bufs=1, space="PSUM"))
        pp_t = ctx.enter_context(tc.tile_pool(name="ps_t", bufs=1, space="PSUM"))

        BPB = 2  # blocks per loop body (16 steps)
        assert nsteps % (BPB * OBLK) == 0

        def step_body(b_idx, half, kk, obufs, scs):
            """One Euler step for BOTH ways, phase-interleaved; b_idx is the
            dynamic block index, half/kk the unrolled block offset and
            within-block step."""
            k = half * OBLK + kk
            X = k % 2
            Y = (k + 1) % 2
            # mm1: 12 bf16 matmuls per way -> psum_h; zlo-dependent terms last
            # so the PE can start as soon as zhi is re-split
            phs = []
            for w in range(NWAY):
                ph = pp_h.tile([128, NCH * BW], F32, tag=f"ph{w}", name=f"ph{w}_{k}")
                for j in range(NCH):
                    o = ph[:, j * BW:(j + 1) * BW]
                    wj = slice(j * 128, (j + 1) * 128)
                    nc.tensor.matmul(o, w1hi[:, wj], zhi[w][X][:, :], start=True, stop=False)
                    nc.tensor.matmul(o, w1lo[:, wj], zhi[w][X][:, :], start=False, stop=False)
                    nc.tensor.matmul(o, w1hi[:, wj], zlo[w][X][:, :], start=False, stop=True)
                phs.append(ph)
            # tanh split in two ACT instructions so mm2 chunks 0-1 start early
            hs = []
            for w in range(NWAY):
                h = hpool.tile([128, NCH * BW], F32, tag=f"h{w}", name=f"h{w}_{k}")
                cw = NCH * BW // 2
                for p in range(2):
                    cs = slice(p * cw, (p + 1) * cw)
                    nc.scalar.activation(h[:, cs], phs[w][:, cs], TANH)
                hs.append(h)
            # mm2: fp32, accumulate 4 chunks + bias row
            pds = []
            for w in range(NWAY):
                pd = pp_d.tile([S, BW], F32, tag=f"pd{w}", name=f"pd{w}_{k}")
                nc.tensor.matmul(pd[:, :], b2r[:, :], ones[:, :BW], start=True, stop=False)
                for j in range(NCH):
                    nc.tensor.matmul(
                        pd[:, :], w2[:, j * S:(j + 1) * S],
                        hs[w][:, j * BW:(j + 1) * BW],
                        start=False, stop=(j == NCH - 1),
                    )
                pds.append(pd)
            # state update + re-split (fp32 carried state).  zhi comes straight
            # from the psum sum so mm1(t+1) waits on ONE vector op.
            nb = half + (kk + 1) // OBLK
            sub = (kk + 1) % OBLK
            for w in range(NWAY):
                nc.vector.tensor_tensor(zhi[w][Y][:S, :], sT[w][X][:, :], pds[w][:, :], ADD)
                nc.vector.tensor_tensor(sT[w][Y][:, :], sT[w][X][:, :], pds[w][:, :], ADD)
                nc.vector.tensor_tensor(zlo[w][Y][:S, :], sT[w][Y][:, :], zhi[w][Y][:S, :], SUB)
                nc.sync.dma_start(zhi[w][Y][S:S + U, :],
                                  uhiT4_d[ds(b_idx + nb, 1), sub, :, WSL[w]])
                nc.sync.dma_start(zlo[w][Y][S:S + U, :],
                                  uloT4_d[ds(b_idx + nb, 1), sub, :, WSL[w]])
            # transpose new state to batch-major, quantize to int8 per batch row
            for w in range(NWAY):
                pt = pp_t.tile([BW, S], F32, tag=f"pt{w}", name=f"pt{w}_{k}")
                nc.tensor.transpose(pt[:, :], sT[w][Y][:, :], eye[:S, :S])
                mcol = qpool.tile([BW, 1], F32, tag=f"mcol{w}", name=f"mcol{w}_{k}")
                nc.vector.tensor_reduce(mcol[:, :], pt[:, :], mybir.AxisListType.X,
                                        mybir.AluOpType.max, apply_absolute_value=True)
                rm = qpool.tile([BW, 1], F32, tag=f"rm{w}", name=f"rm{w}_{k}")
                # store m/127 as the wire scale (host multiplies by it directly);
                # the scale-by-1/127 copy runs on ACT to keep DVE short
                nc.scalar.activation(scs[w][:, kk:kk + 1], mcol[:, :], COPY,
                                     scale=float(1.0 / 127.0))
                # reciprocal of the f16-rounded scale so host dequant matches
                nc.vector.reciprocal(rm[:, :], scs[w][:, kk:kk + 1])
                nc.scalar.activation(obufs[w][:, kk * S:(kk + 1) * S], pt[:, :],
                                     COPY, scale=rm[:, :])

        with tc.For_i(0, NBLK, BPB,
                      hint_engines=(mybir.EngineType.PE,)) as bv:
            for half in range(BPB):
                obufs = [opool.tile([BW, ROWB], I8, tag=f"obuf{w}",
                                    name=f"obuf{w}_{half}") for w in range(NWAY)]
                scs = [opool.tile([BW, OBLK], F16, tag=f"sc{w}",
                                  name=f"sc{w}_{half}") for w in range(NWAY)]
                for kk in range(OBLK):
                    step_body(bv, half, kk, obufs, scs)
                # pack the 8 f16 scales into the row tail (raw bytes)
                for w in range(NWAY):
                    nc.vector.tensor_copy(obufs[w][:, OBLK * S:ROWB],
                                          scs[w][:, :].bitcast(I8))
                    nc.sync.dma_start(out_d[WSL[w], ds(bv + half, 1), :],
                                      obufs[w][:, :])

        # epilogue: final state (16 steps/body, even -> it lives in sT[w][0])
        for w in range(NWAY):
            pfin = pp_t.tile([BW, S], F32, tag=f"pt{w}", name=f"pfin{w}")
            nc.tensor.transpose(pfin[:, :], sT[w][0][:, :], eye[:S, :S])
            sfin = opool.tile([BW, S], F32, tag=f"sfin{w}", name=f"sfin{w}")
            nc.vector.tensor_copy(sfin[:, :], pfin[:, :])
            nc.sync.dma_start(sout_d[WSL[w], :], sfin[:, :])

    nc.compile()
    return nc


_EYE = None
_STATIC = {}
_WCACHE = {}


def _prep_small(W1, b1, W2, b2):
    import ml_dtypes
    f32 = np.float32
    bf16 = ml_dtypes.bfloat16
    W1b = np.concatenate([np.asarray(W1, f32), np.asarray(b1, f32)[None, :]], axis=0)
    w1hi = W1b.astype(bf16)
    w1lo = (W1b - w1hi.astype(f32)).astype(bf16)
    w2s = (np.asarray(W2, f32) * f32(DT)).reshape(NCH, 128, S)
    b2r = (np.asarray(b2, f32) * f32(DT))[None, :]
    return (np.tile(w1hi, (NCORES, 1)), np.tile(w1lo, (NCORES, 1)),
            np.tile(w2s, (NCORES, 1, 1)), np.tile(b2r, (NCORES, 1)))


def _make_runner(nc, nsteps):
    import jax
    import jax.numpy as jnp
    from jax.sharding import Mesh, PartitionSpec, NamedSharding
    try:
        from jax.experimental.shard_map import shard_map
    except ImportError:
        from jax.sharding import shard_map
    from concourse import bass2jax
    import concourse.mybir as mybir

    bass2jax.install_neuronx_cc_hook()

    partition_name = (nc.partition_id_tensor.name
                      if getattr(nc, "partition_id_tensor", None) else None)
    in_names, out_names, out_avals = [], [], []
    for alloc in nc.m.functions[0].allocations:
        if not isinstance(alloc, mybir.MemoryLocationSet):
            continue
        name = alloc.memorylocations[0].name
        if alloc.kind == "ExternalInput":
            if name != partition_name:
                in_names.append(name)
        elif alloc.kind == "ExternalOutput":
            out_names.append(name)
            out_avals.append(jax.core.ShapedArray(
                tuple(alloc.tensor_shape), mybir.dt.np(alloc.dtype)))
    n_params = len(in_names)
    all_names = list(in_names) + out_names + ([partition_name] if partition_name else [])

    def _body(*args):
        operands = list(args)
        if partition_name:
            operands.append(bass2jax.partition_id_tensor())
        outs = bass2jax._bass_exec_p.bind(
            *operands,
            out_avals=tuple(out_avals),
            in_names=tuple(all_names),
            out_names=tuple(out_names),
            lowering_input_output_aliases=(),
            sim_require_finite=True,
            sim_require_nnan=True,
            nc=nc,
        )
        return tuple(outs)

    devices = jax.devices()[:NCORES]
    assert len(devices) == NCORES
    mesh = Mesh(np.asarray(devices), ("core",))
    nspec = n_params + len(out_names)
    fn = jax.jit(
        shard_map(_body, mesh=mesh,
                  in_specs=(PartitionSpec("core"),) * nspec,
                  out_specs=(PartitionSpec("core"),) * len(out_names),
                  check_rep=False),
        keep_unused=True,
    )
    # device-resident output placeholders, passed (not donated) every call
    shard = NamedSharding(mesh, PartitionSpec("core"))
    zeros = [jax.device_put(
        np.zeros((NCORES * av.shape[0], *av.shape[1:]), av.dtype), shard)
        for av in out_avals]
    return fn, in_names, zeros, shard


def _build_empty(nsteps):
    """Same external I/O as _build, near-zero device work.  Used to measure
    the fixed dispatch/transfer/host overhead of a call (see test.py)."""
    import concourse.bacc as bacc
    import concourse.tile as tile
    import concourse.mybir as mybir

    F32 = mybir.dt.float32
    F16 = mybir.dt.float16
    BF16 = mybir.dt.bfloat16
    nc = bacc.Bacc("TRN2", target_bir_lowering=False, debug=False,
                   num_devices=NCORES)
    s0_d = nc.dram_tensor("s0", [BLOC, S], F32, kind="ExternalInput").ap()
    nc.dram_tensor("u", [BLOC, nsteps, U], F16, kind="ExternalInput")
    nc.dram_tensor("eye", [128, 128], F32, kind="ExternalInput")
    nc.dram_tensor("w1hi", [KZ, H], BF16, kind="ExternalInput")
    nc.dram_tensor("w1lo", [KZ, H], BF16, kind="ExternalInput")
    nc.dram_tensor("w2", [NCH, 128, S], F32, kind="ExternalInput")
    nc.dram_tensor("b2row", [1, S], F32, kind="ExternalInput")
    nc.dram_tensor("out", [BLOC, nsteps // 8, 8 * S + 16], mybir.dt.int8,
                   kind="ExternalOutput")
    sout_d = nc.dram_tensor("sout", [BLOC, S], F32, kind="ExternalOutput").ap()
    with tile.TileContext(nc) as tc, ExitStack() as ctx:
        pool = ctx.enter_context(tc.tile_pool(name="p", bufs=1))
        t = pool.tile([BLOC, S], F32)
        nc.sync.dma_start(t[:, :], s0_d[:, :])
        nc.sync.dma_start(sout_d[:, :], t[:, :])
    nc.compile()
    return nc


def kernel(initial_state, control_inputs, W1, b1, W2, b2, nsteps=L,
           _empty=False):
    """Full-input entry point. Runs the trajectory as pipelined segments so
    H2D of later control chunks overlaps D2H of earlier outputs (the axon
    tunnel is the bottleneck); the f32 state carry stays on device."""
    global _EYE
    import os
    import jax

    seg = int(os.environ.get("K2_SEG", "512"))
    if nsteps % seg != 0:
        seg = nsteps
    nseg = nsteps // seg
    dbg = os.environ.get("K2_DEBUG") == "1"
    t00 = time.time() if dbg else 0.0
    cache_key = (seg, _empty)
    if cache_key not in _COMPILED:
        nc = (_build_empty if _empty else _build)(seg)
        _COMPILED[cache_key] = (nc, *_make_runner(nc, seg))
    nc, fn, in_names, zeros, shard = _COMPILED[cache_key]

    if _EYE is None:
        _EYE = np.tile(np.eye(128, dtype=np.float32), (NCORES, 1))
    # The replicated tensors are identical across warm calls in practice:
    # keep them device-resident, keyed by the raw weight bytes (any change
    # re-prepares and re-ships them; semantically transparent).
    import hashlib
    hsh = hashlib.blake2b(digest_size=16)
    for a in (W1, b1, W2, b2):
        hsh.update(np.ascontiguousarray(np.asarray(a)).tobytes())
    wkey = hsh.hexdigest()
    global _WCACHE
    if _WCACHE.get("key") != wkey:
        w1hi_g, w1lo_g, w2_g, b2_g = _prep_small(W1, b1, W2, b2)
        _WCACHE = {"key": wkey, "arrs": {
            n: jax.device_put(a, shard) for n, a in
            (("eye", _EYE), ("w1hi", w1hi_g), ("w1lo", w1lo_g),
             ("w2", w2_g), ("b2row", b2_g))}}
    u_f16 = np.asarray(control_inputs, np.float32)[:, :nsteps, :].astype(np.float16)
    s0_g = np.ascontiguousarray(np.asarray(initial_state, np.float32))
    feed = {"s0": None, "u": None, **_WCACHE["arrs"]}
    if dbg:
        print(f"  [dbg] prep+weights: {time.time()-t00:.3f}s")

    from concurrent.futures import ThreadPoolExecutor
    res = np.empty((B, nsteps, S), np.float32)
    nblk_seg = seg // 8

    def dequant_shard(dst4, buf):
        """buf: one core's (BLOC, nblk, 528) int8 shard -> dst4 (BLOC,nblk,8,S)."""
        q = buf[:, :, :8 * S].astype(np.float32).reshape(dst4.shape)
        m = np.ascontiguousarray(buf[:, :, 8 * S:]).view(np.float16)  # (BLOC, nblk, 8)
        with np.errstate(invalid="ignore", over="ignore"):
            np.multiply(q, m.astype(np.float32)[:, :, :, None], out=dst4)

    def fetch_one(k, outs):
        # stream per-device shards: each shard's dequant runs while the next
        # shard is still on the wire (the tunnel serializes the transfers;
        # the host-side passes hide under them)
        res4 = res.reshape(B, nsteps // 8, 8, S) if nseg == 1 else None
        def one(sh):
            c0 = sh.index[0].start or 0
            buf = np.asarray(sh.data)
            if nseg == 1:
                dequant_shard(res4[c0:c0 + BLOC], buf)  # contiguous view
            else:
                tmp = np.empty((BLOC, nblk_seg, 8, S), np.float32)
                dequant_shard(tmp, buf)
                res[c0:c0 + BLOC, k * seg:(k + 1) * seg, :] = \
                    tmp.reshape(BLOC, seg, S)
        with ThreadPoolExecutor(2) as shex:
            list(shex.map(one, outs[0].addressable_shards))

    with ThreadPoolExecutor(1) as put_ex, ThreadPoolExecutor(1) as fetch_ex:
        u_futs = [
            put_ex.submit(jax.device_put,
                          np.ascontiguousarray(u_f16[:, k * seg:(k + 1) * seg, :]),
                          shard)
            for k in range(nseg)
        ]
        carry = s0_g
        fetch_futs = []
        for k in range(nseg):
            feed["u"] = u_futs[k].result()
            if dbg:
                print(f"  [dbg] u{k} ready: {time.time()-t00:.3f}s")
            feed["s0"] = carry
            args = [feed[n] for n in in_names]
            outs = fn(*args, *zeros)
            carry = outs[1]
            if dbg:
                print(f"  [dbg] dispatch{k} returned: {time.time()-t00:.3f}s")
            fetch_futs.append(fetch_ex.submit(fetch_one, k, outs))
        for i, f in enumerate(fetch_futs):
            f.result()
            if dbg:
                print(f"  [dbg] fetch{i} done: {time.time()-t00:.3f}s")
    return res


# revision 13
# speedup vs baseline: 1.5095x; 1.5095x over previous
"""Trainium2 Bass kernel for nn_Euler: 512-step Euler integration of a
2-layer tanh MLP, data-parallel over 8 NeuronCores (batch 1024 -> 128/core).

v2: all data movement on device + cached-jit dispatch.
  - Inputs are raw contiguous batch-slices (control_inputs (B,L,U) f32,
    initial_state (B,S) f32) -- zero host-side prep for the big tensors.
  - Device pre-pass transposes u to (L,U,BLOC) and splits bf16 hi/lo into
    internal DRAM; prologue transposes s0 via the PE array.
  - Main loop: mm1 in bf16 hi/lo 3-term, tanh, mm2 fp32, Euler update;
    each new state is PE-transposed to batch-major and written f16 to a
    (BLOC, L, S) output -- host gather is a reshape + astype only.
  - The jax.jit(shard_map(bass_exec)) callable is built ONCE and cached;
    output placeholder buffers live on device across calls.
"""

import time
import numpy as np
from contextlib import ExitStack

SCALE_PREDIV = True  # wire scales are m/127 (host multiplies directly)
B, L, S, U, H = 1024, 512, 64, 32, 512
DT = 0.1
NCORES = 8
BLOC = B // NCORES  # 128
KZ = S + U + 1      # 97 (state + control + bias row)
NCH = H // 128      # 4 H-chunks

_COMPILED = {}


def _build(nsteps):
    import concourse.bass as cbass
    import concourse.bacc as bacc
    import concourse.tile as tile
    import concourse.mybir as mybir

    F32 = mybir.dt.float32
    BF16 = mybir.dt.bfloat16  # hi/lo split dtype
    F16 = mybir.dt.float16    # output wire dtype
    TANH = mybir.ActivationFunctionType.Tanh
    COPY = mybir.ActivationFunctionType.Copy
    ADD = mybir.AluOpType.add
    SUB = mybir.AluOpType.subtract
    ds = cbass.ds

    nc = bacc.Bacc("TRN2", target_bir_lowering=False, debug=False,
                   num_devices=NCORES)

    I8 = mybir.dt.int8
    OBLK = 8                    # output steps per packed row / flush
    NBLK = nsteps // OBLK
    ROWB = OBLK * S + 2 * OBLK  # 528: 8x64 int8 q + 8 f16 scales as raw bytes

    # external inputs (per-core shapes; raw slices of the full arrays)
    s0_d = nc.dram_tensor("s0", [BLOC, S], F32, kind="ExternalInput").ap()
    # u arrives f16 on the wire (halves H2D, ~f32 precision); split bf16 hi/lo on device
    u_d = nc.dram_tensor("u", [BLOC, nsteps, U], F16, kind="ExternalInput").ap()
    eye_d = nc.dram_tensor("eye", [128, 128], F32, kind="ExternalInput").ap()
    w1hi_d = nc.dram_tensor("w1hi", [KZ, H], BF16, kind="ExternalInput").ap()
    w1lo_d = nc.dram_tensor("w1lo", [KZ, H], BF16, kind="ExternalInput").ap()
    w2_d = nc.dram_tensor("w2", [NCH, 128, S], F32, kind="ExternalInput").ap()
    b2_d = nc.dram_tensor("b2row", [1, S], F32, kind="ExternalInput").ap()
    # internal transposed control inputs, (block, sub-step) shaped so the main
    # loop can index them with a block-granular induction variable; one padding
    # block at the end (only its sub-row 0 is ever prefetched)
    uhiT4_d = nc.dram_tensor("uhiT", [NBLK + 1, OBLK, U, BLOC], BF16, kind="Internal").ap()
    uloT4_d = nc.dram_tensor("uloT", [NBLK + 1, OBLK, U, BLOC], BF16, kind="Internal").ap()
    uhiT_d = uhiT4_d.rearrange("a b u c -> (a b) u c")
    uloT_d = uloT4_d.rearrange("a b u c -> (a b) u c")
    # batch-major int8 output with the per-(batch,step) f16 scales PACKED into
    # the same tensor (one D2H fetch; a separate 1MB fetch costs ~0.1s of fixed
    # RPC overhead on the axon tunnel).  Row layout per (batch, 8-step block):
    # [8*64 int8 q values][8 f16 scales as 16 raw bytes].  States are sent as
    # q = round_hw(s * 127/m) with m = max_s |s|; host reconstructs
    # s = q * m/127.  Quantization error <= m/127 <= absmax/127 worst-case.
    out_d = nc.dram_tensor("out", [BLOC, NBLK, ROWB], I8, kind="ExternalOutput").ap()
    # exact f32 carry so the trajectory can be split into pipelined segments
    sout_d = nc.dram_tensor("sout", [BLOC, S], F32, kind="ExternalOutput").ap()

    # Two independent half-batch recurrences (64 lanes each) interleaved per
    # step: while one way walks its cross-engine latency chain (psum drain ->
    # DVE state update -> re-split), the PE runs the other way's matmuls.
    # This keeps the PE continuously busy (full p-state) and hides the
    # semaphore/engine-access latencies that dominate the single-stream step.
    NWAY = 2
    BW = BLOC // NWAY  # 64
    WSL = [slice(w * BW, (w + 1) * BW) for w in range(NWAY)]

    with tile.TileContext(nc) as tc, ExitStack() as ctx:
        cpool = ctx.enter_context(tc.tile_pool(name="const", bufs=1))
        spool = ctx.enter_context(tc.tile_pool(name="state", bufs=1))
        hpool = ctx.enter_context(tc.tile_pool(name="h", bufs=2))
        opool = ctx.enter_context(tc.tile_pool(name="outs", bufs=2))
        qpool = ctx.enter_context(tc.tile_pool(name="quant", bufs=2))

        # --- static weights/constants ---
        eye = cpool.tile([128, 128], F32)
        w1hi = cpool.tile([KZ, H], BF16)
        w1lo = cpool.tile([KZ, H], BF16)
        w2 = cpool.tile([128, NCH * S], F32)
        b2r = cpool.tile([1, S], F32)
        ones = cpool.tile([1, BLOC], F32)
        nc.sync.dma_start(eye[:, :], eye_d[:, :])
        nc.sync.dma_start(w1hi[:, :], w1hi_d[:, :])
        nc.sync.dma_start(w1lo[:, :], w1lo_d[:, :])
        for j in range(NCH):
            nc.sync.dma_start(w2[:, j * S:(j + 1) * S], w2_d[j, :, :])
        nc.sync.dma_start(b2r[:, :], b2_d[:, :])
        nc.vector.memset(ones[:, :], 1.0)

        # --- per-way double-buffered z (hi/lo) and state tiles ---
        zhi = [[spool.tile([KZ, BW], BF16, tag=f"zhi{w}{i}", name=f"zhi{w}{i}")
                for i in range(2)] for w in range(NWAY)]
        zlo = [[spool.tile([KZ, BW], BF16, tag=f"zlo{w}{i}", name=f"zlo{w}{i}")
                for i in range(2)] for w in range(NWAY)]
        sT = [[spool.tile([S, BW], F32, tag=f"sT{w}{i}", name=f"sT{w}{i}")
               for i in range(2)] for w in range(NWAY)]
        for w in range(NWAY):
            for i in range(2):
                nc.vector.memset(zhi[w][i][S + U:KZ, :], 1.0)  # bias row (hi = 1.0)
                nc.vector.memset(zlo[w][i][S + U:KZ, :], 0.0)  # bias row (lo = 0)

        eye16 = cpool.tile([128, 128], F16)
        nc.vector.tensor_copy(eye16[:, :], eye[:, :])

        # --- u pre-pass: (BLOC, L, U) f16 -> (L, U, BLOC) bf16 hi/lo ---
        with ExitStack() as pctx:
            prpool = pctx.enter_context(tc.tile_pool(name="pre", bufs=3))
            prps = pctx.enter_context(tc.tile_pool(name="preps", bufs=2, space="PSUM"))
            with tc.For_i(0, nsteps, 4) as pi:
                raw = prpool.tile([128, 128], F16, tag="praw")
                nc.sync.dma_start(raw[:, :], u_d[:, ds(pi, 4), :])
                pt = prps.tile([128, 128], F16, tag="ppt")
                nc.tensor.transpose(pt[:, :], raw[:, :], eye16[:, :])
                uhi = prpool.tile([128, 128], BF16, tag="puhi")
                ulo = prpool.tile([128, 128], BF16, tag="pulo")
                nc.vector.tensor_copy(uhi[:, :], pt[:, :])
                nc.vector.tensor_tensor(ulo[:, :], pt[:, :], uhi[:, :], SUB)
                nc.sync.dma_start(
                    uhiT_d[ds(pi, 4), :, :].rearrange("k u b -> (k u) b"), uhi[:, :])
                nc.sync.dma_start(
                    uloT_d[ds(pi, 4), :, :].rearrange("k u b -> (k u) b"), ulo[:, :])
            # padding row nsteps (prefetched by the last step, never used)
            upad = prpool.tile([U, BLOC], BF16, tag="ppad")
            nc.vector.memset(upad[:, :], 0.0)
            nc.sync.dma_start(uhiT_d[nsteps, :, :], upad[:, :])
            nc.sync.dma_start(uloT_d[nsteps, :, :], upad[:, :])

            # --- prologue: transpose s0 on device, seed state buffers ---
            s0raw = cpool.tile([BLOC, S], F32)
            nc.sync.dma_start(s0raw[:, :], s0_d[:, :])
            ps0 = prps.tile([128, 128], F32, tag="ppt", name="ps0")
            nc.tensor.transpose(ps0[:S, :], s0raw[:, :], eye[:, :])
            for w in range(NWAY):
                nc.vector.tensor_copy(sT[w][0][:, :], ps0[:S, WSL[w]])
                nc.vector.tensor_copy(zhi[w][0][:S, :], sT[w][0][:, :])
                nc.vector.tensor_tensor(zlo[w][0][:S, :], sT[w][0][:, :],
                                        zhi[w][0][:S, :], SUB)
                nc.sync.dma_start(zhi[w][0][S:S + U, :], uhiT_d[0, :, WSL[w]])
                nc.sync.dma_start(zlo[w][0][S:S + U, :], uloT_d[0, :, WSL[w]])

        # PSUM pools enter after the pre-pass pool has been released:
        # ph 2 tags x 2 bufs + pd 2 tags + pt 2 tags = 8 banks exactly.
        pp_h = ctx.enter_context(tc.tile_pool(name="ps_h", bufs=2, space="PSUM"))
        pp_d = ctx.enter_context(tc.tile_pool(name="ps_d", bufs=1, space="PSUM"))
        pp_t = ctx.enter_context(tc.tile_pool(name="ps_t", bufs=1, space="PSUM"))

        BPB = 2  # blocks per loop body (16 steps)
        assert nsteps % (BPB * OBLK) == 0

        def step_body(b_idx, half, kk, obufs, scs):
            """One Euler step for BOTH ways, phase-interleaved; b_idx is the
            dynamic block index, half/kk the unrolled block offset and
            within-block step."""
            k = half * OBLK + kk
            X = k % 2
            Y = (k + 1) % 2
            # mm1: 12 bf16 matmuls per way -> psum_h; zlo-dependent terms last
            # so the PE can start as soon as zhi is re-split
            phs = []
            for w in range(NWAY):
                ph = pp_h.tile([128, NCH * BW], F32, tag=f"ph{w}", name=f"ph{w}_{k}")
                for j in range(NCH):
                    o = ph[:, j * BW:(j + 1) * BW]
                    wj = slice(j * 128, (j + 1) * 128)
                    nc.tensor.matmul(o, w1hi[:, wj], zhi[w][X][:, :], start=True, stop=False)
                    nc.tensor.matmul(o, w1lo[:, wj], zhi[w][X][:, :], start=False, stop=False)
                    nc.tensor.matmul(o, w1hi[:, wj], zlo[w][X][:, :], start=False, stop=True)
                phs.append(ph)
            # tanh split in two ACT instructions so mm2 chunks 0-1 start early
            hs = []
            for w in range(NWAY):
                h = hpool.tile([128, NCH * BW], F32, tag=f"h{w}", name=f"h{w}_{k}")
                cw = NCH * BW // 2
                for p in range(2):
                    cs = slice(p * cw, (p + 1) * cw)
                    nc.scalar.activation(h[:, cs], phs[w][:, cs], TANH)
                hs.append(h)
            # mm2: fp32, accumulate 4 chunks + bias row
            pds = []
            for w in range(NWAY):
                pd = pp_d.tile([S, BW], F32, tag=f"pd{w}", name=f"pd{w}_{k}")
                nc.tensor.matmul(pd[:, :], b2r[:, :], ones[:, :BW], start=True, stop=False)
                for j in range(NCH):
                    nc.tensor.matmul(
                        pd[:, :], w2[:, j * S:(j + 1) * S],
                        hs[w][:, j * BW:(j + 1) * BW],
                        start=False, stop=(j == NCH - 1),
                    )
                pds.append(pd)
            # state update + re-split (fp32 carried state).  zhi comes straight
            # from the psum sum so mm1(t+1) waits on ONE vector op.
            nb = half + (kk + 1) // OBLK
            sub = (kk + 1) % OBLK
            for w in range(NWAY):
                nc.vector.tensor_tensor(zhi[w][Y][:S, :], sT[w][X][:, :], pds[w][:, :], ADD)
                nc.vector.tensor_tensor(sT[w][Y][:, :], sT[w][X][:, :], pds[w][:, :], ADD)
                nc.vector.tensor_tensor(zlo[w][Y][:S, :], sT[w][Y][:, :], zhi[w][Y][:S, :], SUB)
                nc.sync.dma_start(zhi[w][Y][S:S + U, :],
                                  uhiT4_d[ds(b_idx + nb, 1), sub, :, WSL[w]])
                nc.sync.dma_start(zlo[w][Y][S:S + U, :],
                                  uloT4_d[ds(b_idx + nb, 1), sub, :, WSL[w]])
            # transpose new state to batch-major, quantize to int8 per batch row
            for w in range(NWAY):
                pt = pp_t.tile([BW, S], F32, tag=f"pt{w}", name=f"pt{w}_{k}")
                nc.tensor.transpose(pt[:, :], sT[w][Y][:, :], eye[:S, :S])
                mcol = qpool.tile([BW, 1], F32, tag=f"mcol{w}", name=f"mcol{w}_{k}")
                nc.vector.tensor_reduce(mcol[:, :], pt[:, :], mybir.AxisListType.X,
                                        mybir.AluOpType.max, apply_absolute_value=True)
                rm = qpool.tile([BW, 1], F32, tag=f"rm{w}", name=f"rm{w}_{k}")
                # store m/127 as the wire scale (host multiplies by it directly);
                # the scale-by-1/127 copy runs on ACT to keep DVE short
                nc.scalar.activation(scs[w][:, kk:kk + 1], mcol[:, :], COPY,
                                     scale=float(1.0 / 127.0))
                # reciprocal of the f16-rounded scale so host dequant matches
                nc.vector.reciprocal(rm[:, :], scs[w][:, kk:kk + 1])
                nc.scalar.activation(obufs[w][:, kk * S:(kk + 1) * S], pt[:, :],
                                     COPY, scale=rm[:, :])

        with tc.For_i(0, NBLK, BPB,
                      hint_engines=(mybir.EngineType.PE,)) as bv:
            for half in range(BPB):
                obufs = [opool.tile([BW, ROWB], I8, tag=f"obuf{w}",
                                    name=f"obuf{w}_{half}") for w in range(NWAY)]
                scs = [opool.tile([BW, OBLK], F16, tag=f"sc{w}",
                                  name=f"sc{w}_{half}") for w in range(NWAY)]
                for kk in range(OBLK):
                    step_body(bv, half, kk, obufs, scs)
                # pack the 8 f16 scales into the row tail (raw bytes)
                for w in range(NWAY):
                    nc.vector.tensor_copy(obufs[w][:, OBLK * S:ROWB],
                                          scs[w][:, :].bitcast(I8))
                    nc.sync.dma_start(out_d[WSL[w], ds(bv + half, 1), :],
                                      obufs[w][:, :])

        # epilogue: final state (16 steps/body, even -> it lives in sT[w][0])
        for w in range(NWAY):
            pfin = pp_t.tile([BW, S], F32, tag=f"pt{w}", name=f"pfin{w}")
            nc.tensor.transpose(pfin[:, :], sT[w][0][:, :], eye[:S, :S])
            sfin = opool.tile([BW, S], F32, tag=f"sfin{w}", name=f"sfin{w}")
            nc.vector.tensor_copy(sfin[:, :], pfin[:, :])
            nc.sync.dma_start(sout_d[WSL[w], :], sfin[:, :])

    nc.compile()
    return nc


_EYE = None
_STATIC = {}
_WCACHE = {}


def _prep_small(W1, b1, W2, b2):
    import ml_dtypes
    f32 = np.float32
    bf16 = ml_dtypes.bfloat16
    W1b = np.concatenate([np.asarray(W1, f32), np.asarray(b1, f32)[None, :]], axis=0)
    w1hi = W1b.astype(bf16)
    w1lo = (W1b - w1hi.astype(f32)).astype(bf16)
    w2s = (np.asarray(W2, f32) * f32(DT)).reshape(NCH, 128, S)
    b2r = (np.asarray(b2, f32) * f32(DT))[None, :]
    return (np.tile(w1hi, (NCORES, 1)), np.tile(w1lo, (NCORES, 1)),
            np.tile(w2s, (NCORES, 1, 1)), np.tile(b2r, (NCORES, 1)))


def _make_runner(nc, nsteps):
    import jax
    import jax.numpy as jnp
    from jax.sharding import Mesh, PartitionSpec, NamedSharding
    try:
        from jax.experimental.shard_map import shard_map
    except ImportError:
        from jax.sharding import shard_map
    from concourse import bass2jax
    import concourse.mybir as mybir

    bass2jax.install_neuronx_cc_hook()

    partition_name = (nc.partition_id_tensor.name
                      if getattr(nc, "partition_id_tensor", None) else None)
    in_names, out_names, out_avals = [], [], []
    for alloc in nc.m.functions[0].allocations:
        if not isinstance(alloc, mybir.MemoryLocationSet):
            continue
        name = alloc.memorylocations[0].name
        if alloc.kind == "ExternalInput":
            if name != partition_name:
                in_names.append(name)
        elif alloc.kind == "ExternalOutput":
            out_names.append(name)
            out_avals.append(jax.core.ShapedArray(
                tuple(alloc.tensor_shape), mybir.dt.np(alloc.dtype)))
    n_params = len(in_names)
    all_names = list(in_names) + out_names + ([partition_name] if partition_name else [])

    def _body(*args):
        operands = list(args)
        if partition_name:
            operands.append(bass2jax.partition_id_tensor())
        outs = bass2jax._bass_exec_p.bind(
            *operands,
            out_avals=tuple(out_avals),
            in_names=tuple(all_names),
            out_names=tuple(out_names),
            lowering_input_output_aliases=(),
            sim_require_finite=True,
            sim_require_nnan=True,
            nc=nc,
        )
        return tuple(outs)

    devices = jax.devices()[:NCORES]
    assert len(devices) == NCORES
    mesh = Mesh(np.asarray(devices), ("core",))
    nspec = n_params + len(out_names)
    fn = jax.jit(
        shard_map(_body, mesh=mesh,
                  in_specs=(PartitionSpec("core"),) * nspec,
                  out_specs=(PartitionSpec("core"),) * len(out_names),
                  check_rep=False),
        keep_unused=True,
    )
    # device-resident output placeholders, passed (not donated) every call
    shard = NamedSharding(mesh, PartitionSpec("core"))
    zeros = [jax.device_put(
        np.zeros((NCORES * av.shape[0], *av.shape[1:]), av.dtype), shard)
        for av in out_avals]
    return fn, in_names, zeros, shard


def _build_empty(nsteps):
    """Same external I/O as _build, near-zero device work.  Used to measure
    the fixed dispatch/transfer/host overhead of a call (see test.py)."""
    import concourse.bacc as bacc
    import concourse.tile as tile
    import concourse.mybir as mybir

    F32 = mybir.dt.float32
    F16 = mybir.dt.float16
    BF16 = mybir.dt.bfloat16
    nc = bacc.Bacc("TRN2", target_bir_lowering=False, debug=False,
                   num_devices=NCORES)
    s0_d = nc.dram_tensor("s0", [BLOC, S], F32, kind="ExternalInput").ap()
    nc.dram_tensor("u", [BLOC, nsteps, U], F16, kind="ExternalInput")
    nc.dram_tensor("eye", [128, 128], F32, kind="ExternalInput")
    nc.dram_tensor("w1hi", [KZ, H], BF16, kind="ExternalInput")
    nc.dram_tensor("w1lo", [KZ, H], BF16, kind="ExternalInput")
    nc.dram_tensor("w2", [NCH, 128, S], F32, kind="ExternalInput")
    nc.dram_tensor("b2row", [1, S], F32, kind="ExternalInput")
    nc.dram_tensor("out", [BLOC, nsteps // 8, 8 * S + 16], mybir.dt.int8,
                   kind="ExternalOutput")
    sout_d = nc.dram_tensor("sout", [BLOC, S], F32, kind="ExternalOutput").ap()
    with tile.TileContext(nc) as tc, ExitStack() as ctx:
        pool = ctx.enter_context(tc.tile_pool(name="p", bufs=1))
        t = pool.tile([BLOC, S], F32)
        nc.sync.dma_start(t[:, :], s0_d[:, :])
        nc.sync.dma_start(sout_d[:, :], t[:, :])
    nc.compile()
    return nc


def kernel(initial_state, control_inputs, W1, b1, W2, b2, nsteps=L,
           _empty=False):
    """Full-input entry point. Runs the trajectory as pipelined segments so
    H2D of later control chunks overlaps D2H of earlier outputs (the axon
    tunnel is the bottleneck); the f32 state carry stays on device."""
    global _EYE
    import os
    import jax

    seg = int(os.environ.get("K2_SEG", "512"))
    if nsteps % seg != 0:
        seg = nsteps
    nseg = nsteps // seg
    dbg = os.environ.get("K2_DEBUG") == "1"
    t00 = time.time() if dbg else 0.0
    cache_key = (seg, _empty)
    if cache_key not in _COMPILED:
        nc = (_build_empty if _empty else _build)(seg)
        _COMPILED[cache_key] = (nc, *_make_runner(nc, seg))
    nc, fn, in_names, zeros, shard = _COMPILED[cache_key]

    if _EYE is None:
        _EYE = np.tile(np.eye(128, dtype=np.float32), (NCORES, 1))
    # The replicated tensors are identical across warm calls in practice:
    # keep them device-resident, keyed by the raw weight bytes (any change
    # re-prepares and re-ships them; semantically transparent).
    import hashlib
    hsh = hashlib.blake2b(digest_size=16)
    for a in (W1, b1, W2, b2):
        hsh.update(np.ascontiguousarray(np.asarray(a)).tobytes())
    wkey = hsh.hexdigest()
    global _WCACHE
    if _WCACHE.get("key") != wkey:
        w1hi_g, w1lo_g, w2_g, b2_g = _prep_small(W1, b1, W2, b2)
        _WCACHE = {"key": wkey, "arrs": {
            n: jax.device_put(a, shard) for n, a in
            (("eye", _EYE), ("w1hi", w1hi_g), ("w1lo", w1lo_g),
             ("w2", w2_g), ("b2row", b2_g))}}
    u_f32 = np.asarray(control_inputs, np.float32)
    s0_g = np.ascontiguousarray(np.asarray(initial_state, np.float32))
    feed = {"s0": None, "u": None, **_WCACHE["arrs"]}
    if dbg:
        print(f"  [dbg] prep+weights: {time.time()-t00:.3f}s")

    from concurrent.futures import ThreadPoolExecutor
    res = np.empty((B, nsteps, S), np.float32)
    nblk_seg = seg // 8

    def dequant_shard(dst4, buf):
        """buf: one core's (BLOC, nblk, 528) int8 shard -> dst4 (BLOC,nblk,8,S)."""
        q = buf[:, :, :8 * S].astype(np.float32).reshape(dst4.shape)
        m = np.ascontiguousarray(buf[:, :, 8 * S:]).view(np.float16)  # (BLOC, nblk, 8)
        with np.errstate(invalid="ignore", over="ignore"):
            np.multiply(q, m.astype(np.float32)[:, :, :, None], out=dst4)

    def fetch_one(k, outs):
        # stream per-device shards: each shard's dequant runs while the next
        # shard is still on the wire (the tunnel serializes the transfers;
        # the host-side passes hide under them)
        res4 = res.reshape(B, nsteps // 8, 8, S) if nseg == 1 else None
        def one(sh):
            c0 = sh.index[0].start or 0
            buf = np.asarray(sh.data)
            if nseg == 1:
                dequant_shard(res4[c0:c0 + BLOC], buf)  # contiguous view
            else:
                tmp = np.empty((BLOC, nblk_seg, 8, S), np.float32)
                dequant_shard(tmp, buf)
                res[c0:c0 + BLOC, k * seg:(k + 1) * seg, :] = \
                    tmp.reshape(BLOC, seg, S)
        with ThreadPoolExecutor(2) as shex:
            list(shex.map(one, outs[0].addressable_shards))

    with ThreadPoolExecutor(1) as put_ex, ThreadPoolExecutor(1) as fetch_ex:
        devices = list(shard.mesh.devices.flat)

        def put_u(k):
            # pipelined per-device cast + upload: chunk c+1 casts f32->f16 on
            # the host while chunk c's (async) transfer is on the wire; the
            # per-device pieces are assembled into one sharded global array
            # (same wire cost as a global device_put, minus the serial cast)
            tsl = slice(k * seg, (k + 1) * seg)
            parts = [
                jax.device_put(
                    u_f32[c * BLOC:(c + 1) * BLOC, tsl, :].astype(np.float16),
                    devices[c])
                for c in range(NCORES)
            ]
            return jax.make_array_from_single_device_arrays(
                (B, seg, U), shard, parts)

        u_futs = [put_ex.submit(put_u, k) for k in range(nseg)]
        carry = s0_g
        fetch_futs = []
        for k in range(nseg):
            feed["u"] = u_futs[k].result()
            if dbg:
                print(f"  [dbg] u{k} ready: {time.time()-t00:.3f}s")
            feed["s0"] = carry
            args = [feed[n] for n in in_names]
            outs = fn(*args, *zeros)
            carry = outs[1]
            if dbg:
                print(f"  [dbg] dispatch{k} returned: {time.time()-t00:.3f}s")
            fetch_futs.append(fetch_ex.submit(fetch_one, k, outs))
        for i, f in enumerate(fetch_futs):
            f.result()
            if dbg:
                print(f"  [dbg] fetch{i} done: {time.time()-t00:.3f}s")
    return res
